# revision 6
# baseline (speedup 1.0000x reference)
"""Dinov3 ViT attention (B=4, N=1024, D=1024, H=16, HD=64) on 8 TRN2
NeuronCores, written against the Bass/Tile stack.

Sharding: core c -> (batch b = c//2, head-group g = c%2, 8 heads each).
Each core computes q/k/v projections for its 512-feature slice, rotary,
attention, and a partial o_proj (its head-group's wo columns). The host
sums the two partials per batch and adds the constant bias vector
(bo + bv @ wo.T - exact, since softmax rows sum to 1).

v2 schedule (per core, fp16 matmuls / fp32 accumulation):
  - inputs DMA'd in need-order chunks (wv per-k, xt per-token-block in a
    t-major layout) so the first V matmul starts ~2us in, not after the
    full 6.5 MB burst.
  - phase 0: V projections t=0..7; Q(0)/K(0) projections + rotary
    threaded into t=5..7.
  - pair m: scores loop with next pair's Q projection interleaved
    (2 MMs per kb) to cover the ACT exp latency; then AV(par0),
    K(m+1) projection (16 MMs, covers par0's normalization chain),
    AV(par1). Normalization per par: DVE reciprocal on the psum ones-row,
    gpsimd half-broadcast, DVE mult straight from psum -> fp16 ot.
  - pair 3: AV MMs interleaved into the scores loop (no next-pair
    projections to thread); AV(par1) borrows a scores-pool psum tile.
  - o_proj: per token block accumulate pairs 0..2 first; pair-3
    contributions issued after, so the last normalization chain hides
    under 24 ready MMs. fp16 partial output, per-block DMA.
Host passes pre-transposed/sliced fp16 inputs; host sums the two
partials per batch in fp32. PSUM: pq 2 + pss 4 + pav 2 = 8 banks.
"""

import sys

if "/opt/trn_rl_repo" not in sys.path:
    sys.path.insert(0, "/opt/trn_rl_repo")

import numpy as np

import concourse.bass as bass
import concourse.bacc as bacc
import concourse.mybir as mybir
from concourse import tile
from concourse import bass_utils
from contextlib import ExitStack

B, N, D = 4, 1024, 1024
H, HD = 16, 64
F = 512          # per-core feature slice (8 heads)
P = 128
NKB = 8          # contraction blocks over D
NTB = 8          # token blocks of 128
NH = 8           # local heads
MODE = "f16"     # "f16" | "bf16" | "f32r"

_CACHE = {}


def build_nc(mode="f16", debug=False):
    assert mode in ("f16", "bf16", "f32r")
    if mode == "f16":
        dt = mybir.dt.float16
    elif mode == "bf16":
        dt = mybir.dt.bfloat16
    else:
        dt = mybir.dt.float32r
    f32 = mybir.dt.float32
    AF = mybir.ActivationFunctionType
    ALU = mybir.AluOpType

    nc = bacc.Bacc("TRN2", target_bir_lowering=False, debug=False, num_devices=8)
    # xt is t-major: [P, NTB, NKB, P] flattened
    xt_d = nc.dram_tensor("xt", (P, NTB * NKB * P), dt, kind="ExternalInput").ap()
    wqt_d = nc.dram_tensor("wqt", (P, NKB * F), dt, kind="ExternalInput").ap()
    wkt_d = nc.dram_tensor("wkt", (P, NKB * F), dt, kind="ExternalInput").ap()
    wvt_d = nc.dram_tensor("wvt", (P, NKB * F), dt, kind="ExternalInput").ap()
    wot_d = nc.dram_tensor("wot", (P, 4 * D), dt, kind="ExternalInput").ap()
    bq_d = nc.dram_tensor("bq", (P, 4), f32, kind="ExternalInput").ap()
    cs_d = nc.dram_tensor("cs", (P, N), dt, kind="ExternalInput").ap()
    ss_d = nc.dram_tensor("ss", (P, N), dt, kind="ExternalInput").ap()
    out_d = nc.dram_tensor("out", (N, D), dt, kind="ExternalOutput").ap()

    with tile.TileContext(nc) as tc, ExitStack() as top:
        pool = top.enter_context(tc.tile_pool(name="sb", bufs=1))

        cs_sb = pool.tile([P, N], dt, name="cs")
        ss_sb = pool.tile([P, N], dt, name="ss")
        bq_sb = pool.tile([P, 4], f32, name="bq")
        ebias = pool.tile([P, 1], f32, name="ebias")
        xt_big = pool.tile([P, NTB * NKB * P], dt, name="xtb")
        wq_big = pool.tile([P, NKB * F], dt, name="wqb")
        wk_big = pool.tile([P, NKB * F], dt, name="wkb")
        wv_big = pool.tile([P, NKB * F], dt, name="wvb")
        wot_big = pool.tile([P, 4 * D], dt, name="wotb")
        # views
        xt4 = xt_big[:].rearrange("p (t k j) -> p t k j", t=NTB, k=NKB)
        wq_sb = [wq_big[:, k * F:(k + 1) * F] for k in range(NKB)]
        wk_sb = [wk_big[:, k * F:(k + 1) * F] for k in range(NKB)]
        wv_sb = [wv_big[:, k * F:(k + 1) * F] for k in range(NKB)]
        wot_sb = [wot_big[:, m * D:(m + 1) * D] for m in range(4)]
        qt_sb = [pool.tile([P, N], dt, name=f"qt{m}") for m in range(4)]
        kt_sb = [pool.tile([P, N], dt, name=f"kt{m}") for m in range(4)]
        v65_sb = [pool.tile([P, NH * 128], dt, name=f"v65_{t}") for t in range(NTB)]
        ot_sb = [pool.tile([P, N], dt, name=f"ot{m}") for m in range(4)]
        rcp_h = [pool.tile([1, N], f32, name=f"rcp{h}") for h in range(NH)]

        nc.any.memset(ebias[:], -3.0 if mode == "f16" else 0.0)
        # v65 ones-init (cols 0 of each head slot feed the softmax
        # denominator); free during the initial DMA wait
        for t in range(NTB):
            eng = nc.gpsimd if t % 2 == 0 else nc.vector
            eng.memset(v65_sb[t][:], 1.0)

        # ---- input DMAs in need-order chunks ----
        KJ = NKB * P
        engs = [nc.sync, nc.scalar, nc.gpsimd]
        # wv per-k chunks + xt t=0 first (V(t=0) needs wv[k] and xt[t0])
        nc.sync.dma_start(wv_big[:, 0:2 * F], wvt_d[:, 0:2 * F])
        nc.scalar.dma_start(xt_big[:, 0:KJ], xt_d[:, 0:KJ])
        nc.gpsimd.dma_start(wv_big[:, 2 * F:4 * F], wvt_d[:, 2 * F:4 * F])
        nc.sync.dma_start(wv_big[:, 4 * F:6 * F], wvt_d[:, 4 * F:6 * F])
        nc.scalar.dma_start(wv_big[:, 6 * F:8 * F], wvt_d[:, 6 * F:8 * F])
        for t in range(1, NTB):
            engs[t % 3].dma_start(xt_big[:, t * KJ:(t + 1) * KJ],
                                  xt_d[:, t * KJ:(t + 1) * KJ])
        for h in range(2):
            hw = slice(h * 4 * F, (h + 1) * 4 * F)
            nc.sync.dma_start(wq_big[:, hw], wqt_d[:, hw])
            nc.scalar.dma_start(wk_big[:, hw], wkt_d[:, hw])
        nc.gpsimd.dma_start(cs_sb[:], cs_d)
        nc.gpsimd.dma_start(ss_sb[:], ss_d)
        nc.sync.dma_start(bq_sb[:], bq_d)

        swp = top.enter_context(tc.tile_pool(name="swp", bufs=2))
        ptp = top.enter_context(tc.tile_pool(name="ptp", bufs=18))
        rbp = top.enter_context(tc.tile_pool(name="rbp", bufs=2))
        ost = top.enter_context(tc.tile_pool(name="ost", bufs=3))

        pq = top.enter_context(tc.tile_pool(name="pq", bufs=1, space="PSUM"))

        def qk_full(m, which):
            """Full Q or K projection for pair m: 16 MMs, halves alternating."""
            w_sb = wq_sb if which == "q" else wk_sb
            ps = pq.tile([P, N], f32, tag="pq", name="psqk")
            for k in range(NKB):
                for half in range(2):
                    hs = slice(half * F, (half + 1) * F)
                    nc.tensor.matmul(
                        ps[:, hs], w_sb[k][:, m * P:(m + 1) * P],
                        xt4[:, half * 4:half * 4 + 4, k, :],
                        start=(k == 0), stop=(k == NKB - 1))
            return ps

        def qk_store(m, which, ps):
            dst = qt_sb if which == "q" else kt_sb
            if which == "q":
                nc.vector.tensor_scalar_add(dst[m][:], ps[:], bq_sb[:, m:m + 1])
            else:
                nc.vector.tensor_copy(dst[m][:], ps[:])

        def rotary(m, src_sb):
            sw = swp.tile([P, N], dt, tag="sw", name="sw")
            for blk in range(4):
                o = blk * 32
                nc.gpsimd.dma_start(sw[o:o + 32, :],
                                    src_sb[m][o ^ 32:(o ^ 32) + 32, :])
            nc.vector.tensor_tensor(sw[:], sw[:], ss_sb[:], op=ALU.mult)
            nc.vector.tensor_tensor(src_sb[m][:], src_sb[m][:], cs_sb[:],
                                    op=ALU.mult)
            nc.vector.tensor_tensor(src_sb[m][:], src_sb[m][:], sw[:],
                                    op=ALU.add)

        def norm_chain(m, par, av):
            """reciprocal of ones-row, half-broadcast, scale psum -> ot."""
            h = 2 * m + par
            off = par * 64
            nc.vector.reciprocal_approx_fast(rcp_h[h][:], av[0:1, :])
            rb = rbp.tile([64, N], f32, tag="rb", name="rb")
            nc.gpsimd.partition_broadcast(rb[:], rcp_h[h][:])
            nc.vector.tensor_tensor(ot_sb[m][off:off + 64, :], av[64:128, :],
                                    rb[:], op=ALU.mult)

        # ---------- phase 0: V projections; QK(0) threaded into t=5..7 ----
        with ExitStack() as ph0:
            pv = ph0.enter_context(tc.tile_pool(name="pv", bufs=4, space="PSUM"))
            psq0 = psk0 = None
            for t in range(NTB):
                ps = pv.tile([P, F], f32, tag="pv", name="psv")
                for k in range(NKB):
                    nc.tensor.matmul(
                        ps[:], xt4[:, t, k, :], wv_sb[k][:],
                        start=(k == 0), stop=(k == NKB - 1))
                dst = v65_sb[t][:].rearrange("p (h e) -> p h e", e=128)[:, :, 64:128]
                nc.vector.tensor_copy(dst,
                                      ps[:].rearrange("p (h e) -> p h e", e=64))
                if t == 5:
                    psq0 = qk_full(0, "q")
                elif t == 6:
                    qk_store(0, "q", psq0)
                    psk0 = qk_full(0, "k")
                    rotary(0, qt_sb)
                elif t == 7:
                    qk_store(0, "k", psk0)
                    rotary(0, kt_sb)
        nc.sync.dma_start(wot_big[:], wot_d)

        # ---------- attention pairs ----------
        with ExitStack() as phc:
            pss = phc.enter_context(tc.tile_pool(name="pss", bufs=2, space="PSUM"))
            pav = phc.enter_context(tc.tile_pool(name="pav", bufs=1, space="PSUM"))
            for m in range(4):
                nxt = m + 1
                last = nxt >= 4
                pts = ([], [])
                psq = None if last else pq.tile([P, N], f32, tag="pq", name="psq")
                av0 = av1 = None
                for kb in range(NTB):
                    ps2 = [pss.tile([P, N], f32, tag="pss", name="pss")
                           for _ in range(2)]
                    for qh in range(2):
                        qs = slice(qh * F, (qh + 1) * F)
                        for par in range(2):
                            off = par * 64
                            nc.tensor.matmul(
                                ps2[par][:, qs],
                                kt_sb[m][off:off + 64, kb * P:(kb + 1) * P],
                                qt_sb[m][off:off + 64, qs],
                                start=True, stop=True,
                                tile_position=(off, 0))
                    if not last:
                        # thread next pair's Q projection: 2 MMs per kb
                        for half in range(2):
                            hs = slice(half * F, (half + 1) * F)
                            nc.tensor.matmul(
                                psq[:, hs], wq_sb[kb][:, nxt * P:(nxt + 1) * P],
                                xt4[:, half * 4:half * 4 + 4, kb, :],
                                start=(kb == 0), stop=(kb == NKB - 1))
                    else:
                        # last pair: interleave AV MMs (2 kb behind)
                        if kb == 2:
                            av0 = pav.tile([P, N], f32, tag="pav", name="av")
                        if kb >= 2:
                            j = kb - 2
                            for qh in range(2):
                                qs = slice(qh * F, (qh + 1) * F)
                                nc.tensor.matmul(
                                    av0[:, qs],
                                    v65_sb[j][:, (2 * m) * 128:(2 * m + 1) * 128],
                                    pts[0][j][:, qs],
                                    start=(j == 0), stop=False)
                    for par in range(2):
                        ptile = ptp.tile([P, N], dt, tag="pt", name="ptile",
                                         bufs=18)
                        nc.scalar.activation(ptile[:], ps2[par][:], AF.Exp,
                                             scale=0.125, bias=ebias[:])
                        pts[par].append(ptile)

                if not last:
                    qk_store(nxt, "q", psq)
                    rotary(nxt, qt_sb)
                    # AV par0
                    av0 = pav.tile([P, N], f32, tag="pav", name="av")
                    h0 = 2 * m
                    for kb in range(NTB):
                        for qh in range(2):
                            qs = slice(qh * F, (qh + 1) * F)
                            nc.tensor.matmul(
                                av0[:, qs],
                                v65_sb[kb][:, h0 * 128:(h0 + 1) * 128],
                                pts[0][kb][:, qs],
                                start=(kb == 0), stop=(kb == NTB - 1))
                    norm_chain(m, 0, av0)
                    # K(m+1) projection: 16 MMs cover par0's chain + psum drain
                    psk = qk_full(nxt, "k")
                    # AV par1
                    av1 = pav.tile([P, N], f32, tag="pav", name="av")
                    h1 = 2 * m + 1
                    for kb in range(NTB):
                        for qh in range(2):
                            qs = slice(qh * F, (qh + 1) * F)
                            nc.tensor.matmul(
                                av1[:, qs],
                                v65_sb[kb][:, h1 * 128:(h1 + 1) * 128],
                                pts[1][kb][:, qs],
                                start=(kb == 0), stop=(kb == NTB - 1))
                    qk_store(nxt, "k", psk)
                    rotary(nxt, kt_sb)
                    norm_chain(m, 1, av1)
                else:
                    # finish AV par0 (kb 6,7), then par1 from a scores-pool
                    # tile, so par1 need not wait for par0's drain
                    h0, h1 = 2 * m, 2 * m + 1
                    for j in (6,):
                        for qh in range(2):
                            qs = slice(qh * F, (qh + 1) * F)
                            nc.tensor.matmul(
                                av0[:, qs],
                                v65_sb[j][:, h0 * 128:(h0 + 1) * 128],
                                pts[0][j][:, qs], start=False, stop=False)
                    av1 = pss.tile([P, N], f32, tag="pss", name="av1")
                    for j in range(6):
                        for qh in range(2):
                            qs = slice(qh * F, (qh + 1) * F)
                            nc.tensor.matmul(
                                av1[:, qs],
                                v65_sb[j][:, h1 * 128:(h1 + 1) * 128],
                                pts[1][j][:, qs], start=(j == 0), stop=False)
                    for qh in range(2):
                        qs = slice(qh * F, (qh + 1) * F)
                        nc.tensor.matmul(
                            av0[:, qs],
                            v65_sb[7][:, h0 * 128:(h0 + 1) * 128],
                            pts[0][7][:, qs], start=False, stop=True)
                    norm_chain(m, 0, av0)
                    for j in (6, 7):
                        for qh in range(2):
                            qs = slice(qh * F, (qh + 1) * F)
                            nc.tensor.matmul(
                                av1[:, qs],
                                v65_sb[j][:, h1 * 128:(h1 + 1) * 128],
                                pts[1][j][:, qs], start=False, stop=(j == 7))
                    norm_chain(m, 1, av1)

        # ---------- o_proj: pairs 0..2 first, pair 3 contributions last ----
        def oproj_fin(ps, qb):
            for half in range(2):
                hs = slice(half * F, (half + 1) * F)
                nc.tensor.matmul(ps[:, hs], ot_sb[3][:, qb * P:(qb + 1) * P],
                                 wot_sb[3][:, hs], start=False, stop=True)
            ot_out = ost.tile([P, D], dt, tag="oout", name="oout")
            if qb % 2 == 0:
                nc.scalar.copy(ot_out[:], ps[:])
                nc.sync.dma_start(out_d[qb * P:(qb + 1) * P, :], ot_out[:])
            else:
                nc.vector.tensor_copy(ot_out[:], ps[:])
                nc.scalar.dma_start(out_d[qb * P:(qb + 1) * P, :], ot_out[:])

        with ExitStack() as phe:
            po = phe.enter_context(tc.tile_pool(name="po", bufs=3, space="PSUM"))
            pso = []
            for qb in range(NTB):
                ps = po.tile([P, D], f32, tag="po", name="pso", bufs=3)
                pso.append(ps)
                for mm_ in range(3):
                    for half in range(2):
                        hs = slice(half * F, (half + 1) * F)
                        nc.tensor.matmul(
                            ps[:, hs],
                            ot_sb[mm_][:, qb * P:(qb + 1) * P],
                            wot_sb[mm_][:, hs],
                            start=(mm_ == 0), stop=False)
                if qb >= 2:
                    oproj_fin(pso[qb - 2], qb - 2)
            for qb in range(NTB - 2, NTB):
                oproj_fin(pso[qb], qb)

    nc.compile()
    return nc


def host_prep(inputs, mode=MODE):
    """Slice/transpose full inputs into 8 per-core input maps."""
    hs = np.asarray(inputs["hidden_states"], np.float32)
    cos = np.asarray(inputs["cos"], np.float32)
    sin = np.asarray(inputs["sin"], np.float32)
    wq = np.asarray(inputs["wq"], np.float32)
    wk = np.asarray(inputs["wk"], np.float32)
    wv = np.asarray(inputs["wv"], np.float32)
    wo = np.asarray(inputs["wo"], np.float32)
    bq = np.asarray(inputs["bq"], np.float32)

    if mode == "bf16":
        import ml_dtypes
        cast = lambda a: np.ascontiguousarray(a).astype(ml_dtypes.bfloat16)
    elif mode == "f16":
        cast = lambda a: np.ascontiguousarray(a).astype(np.float16)
    else:
        cast = lambda a: np.ascontiguousarray(a, np.float32)

    sgn = np.ones((64, 1), np.float32)
    sgn[:32] = -1.0
    in_maps = []
    for c in range(8):
        b, g = c // 2, c % 2
        fs = slice(g * F, (g + 1) * F)
        csx = cos[b].T  # (64, N)
        ssx = sin[b].T * sgn
        pack = lambda a, kd: np.ascontiguousarray(
            a.reshape(kd, P, -1).transpose(1, 0, 2).reshape(P, -1))
        # xt t-major: [P, t, k, j] from hs[b].T = [k*P+p, t*P+j]
        xtm = hs[b].T.reshape(NKB, P, NTB, P).transpose(1, 2, 0, 3).reshape(P, -1)
        in_maps.append({
            "xt": cast(np.ascontiguousarray(xtm)),
            "wqt": cast(pack(wq[fs, :].T, NKB)),
            "wkt": cast(pack(wk[fs, :].T, NKB)),
            "wvt": cast(pack(wv[fs, :].T, NKB)),
            "wot": cast(pack(wo[:, fs].T, 4)),
            "bq": np.ascontiguousarray(bq[fs].reshape(4, P).T, np.float32),
            "cs": cast(np.concatenate([csx, csx], axis=0)),
            "ss": cast(np.concatenate([ssx, ssx], axis=0)),
        })
    return in_maps


def host_finish(results, inputs):
    bo = np.asarray(inputs["bo"], np.float32)
    bv = np.asarray(inputs["bv"], np.float32)
    wo = np.asarray(inputs["wo"], np.float32)
    const = bo + bv @ wo.T
    out = np.empty((B, N, D), np.float32)
    for b in range(B):
        out[b] = (results[2 * b]["out"].astype(np.float32)
                  + results[2 * b + 1]["out"].astype(np.float32) + const)
    return out


def _get_nc(mode=MODE):
    if mode not in _CACHE:
        _CACHE[mode] = build_nc(mode)
    return _CACHE[mode]


def run(inputs, mode=MODE, trace=False, tmpdir=None):
    nc = _get_nc(mode)
    in_maps = host_prep(inputs, mode)
    res = bass_utils.run_bass_kernel_spmd(
        nc, in_maps, core_ids=list(range(8)), trace=trace, tmpdir=tmpdir)
    return host_finish(res.results, inputs), res


def kernel(**inputs):
    out, _ = run(inputs)
    return out


# revision 9
# speedup vs baseline: 1.0520x; 1.0520x over previous
"""Dinov3 ViT attention (B=4, N=1024, D=1024, H=16, HD=64) on 8 TRN2
NeuronCores, written against the Bass/Tile stack.

Sharding: core c -> (batch b = c//2, head-group g = c%2, 8 heads each).
Each core computes q/k/v projections for its 512-feature slice, rotary,
attention, and a partial o_proj (its head-group's wo columns). The host
sums the two partials per batch and adds the constant bias vector
(bo + bv @ wo.T - exact, since softmax rows sum to 1).

v2 schedule (per core, fp16 matmuls / fp32 accumulation):
  - inputs DMA'd in need-order chunks (wv per-k, xt per-token-block in a
    t-major layout) so the first V matmul starts ~2us in, not after the
    full 6.5 MB burst.
  - phase 0: V projections t=0..7; Q(0)/K(0) projections + rotary
    threaded into t=5..7.
  - pair m: scores loop with next pair's Q projection interleaved
    (2 MMs per kb) to cover the ACT exp latency; then AV(par0),
    K(m+1) projection (16 MMs, covers par0's normalization chain),
    AV(par1). Normalization per par: DVE reciprocal on the psum ones-row,
    gpsimd half-broadcast, DVE mult straight from psum -> fp16 ot.
  - pair 3: AV MMs interleaved into the scores loop (no next-pair
    projections to thread); AV(par1) borrows a scores-pool psum tile.
  - o_proj: per token block accumulate pairs 0..2 first; pair-3
    contributions issued after, so the last normalization chain hides
    under 24 ready MMs. fp16 partial output, per-block DMA.
Host passes pre-transposed/sliced fp16 inputs; host sums the two
partials per batch in fp32. PSUM: pq 2 + pss 4 + pav 2 = 8 banks.
"""

import sys

if "/opt/trn_rl_repo" not in sys.path:
    sys.path.insert(0, "/opt/trn_rl_repo")

import numpy as np

import concourse.bass as bass
import concourse.bacc as bacc
import concourse.mybir as mybir
from concourse import tile
from concourse import bass_utils
from contextlib import ExitStack

B, N, D = 4, 1024, 1024
H, HD = 16, 64
F = 512          # per-core feature slice (8 heads)
P = 128
NKB = 8          # contraction blocks over D
NTB = 8          # token blocks of 128
NH = 8           # local heads
MODE = "f16"     # "f16" | "bf16" | "f32r"

_CACHE = {}


def build_nc(mode="f16", debug=False):
    assert mode in ("f16", "bf16", "f32r")
    if mode == "f16":
        dt = mybir.dt.float16
    elif mode == "bf16":
        dt = mybir.dt.bfloat16
    else:
        dt = mybir.dt.float32r
    f32 = mybir.dt.float32
    AF = mybir.ActivationFunctionType
    ALU = mybir.AluOpType

    nc = bacc.Bacc("TRN2", target_bir_lowering=False, debug=False, num_devices=8)
    # xt is t-major: [P, NTB, NKB, P] flattened
    xt_d = nc.dram_tensor("xt", (P, NTB * NKB * P), dt, kind="ExternalInput").ap()
    wqt_d = nc.dram_tensor("wqt", (P, NKB * F), dt, kind="ExternalInput").ap()
    wkt_d = nc.dram_tensor("wkt", (P, NKB * F), dt, kind="ExternalInput").ap()
    wvt_d = nc.dram_tensor("wvt", (P, NKB * F), dt, kind="ExternalInput").ap()
    wot_d = nc.dram_tensor("wot", (P, 4 * D), dt, kind="ExternalInput").ap()
    bq_d = nc.dram_tensor("bq", (P, 4), f32, kind="ExternalInput").ap()
    cs_d = nc.dram_tensor("cs", (P, N), dt, kind="ExternalInput").ap()
    ss_d = nc.dram_tensor("ss", (P, N), dt, kind="ExternalInput").ap()
    out_d = nc.dram_tensor("out", (N, D), dt, kind="ExternalOutput").ap()

    with tile.TileContext(nc) as tc, ExitStack() as top:
        pool = top.enter_context(tc.tile_pool(name="sb", bufs=1))

        cs_sb = pool.tile([P, N], dt, name="cs")
        ss_sb = pool.tile([P, N], dt, name="ss")
        bq_sb = pool.tile([P, 4], f32, name="bq")
        ebias = pool.tile([P, 1], f32, name="ebias")
        xt_big = pool.tile([P, NTB * NKB * P], dt, name="xtb")
        wq_big = pool.tile([P, NKB * F], dt, name="wqb")
        wk_big = pool.tile([P, NKB * F], dt, name="wkb")
        wv_big = pool.tile([P, NKB * F], dt, name="wvb")
        wot_big = pool.tile([P, 4 * D], dt, name="wotb")
        # views
        xt4 = xt_big[:].rearrange("p (t k j) -> p t k j", t=NTB, k=NKB)
        wq_sb = [wq_big[:, k * F:(k + 1) * F] for k in range(NKB)]
        wk_sb = [wk_big[:, k * F:(k + 1) * F] for k in range(NKB)]
        wv_sb = [wv_big[:, k * F:(k + 1) * F] for k in range(NKB)]
        wot_sb = [wot_big[:, m * D:(m + 1) * D] for m in range(4)]
        qt_sb = [pool.tile([P, N], dt, name=f"qt{m}") for m in range(4)]
        kt_sb = [pool.tile([P, N], dt, name=f"kt{m}") for m in range(4)]
        v65_sb = [pool.tile([P, NH * 128], dt, name=f"v65_{t}") for t in range(NTB)]
        ot_sb = [pool.tile([P, N], dt, name=f"ot{m}") for m in range(4)]
        rcp_h = [pool.tile([1, N], f32, name=f"rcp{h}") for h in range(NH)]

        nc.any.memset(ebias[:], -3.0 if mode == "f16" else 0.0)
        # v65 ones-init (cols 0 of each head slot feed the softmax
        # denominator); free during the initial DMA wait
        for t in range(NTB):
            eng = nc.gpsimd if t % 2 == 0 else nc.vector
            eng.memset(v65_sb[t][:], 1.0)

        # ---- input DMAs in need-order chunks ----
        KJ = NKB * P
        engs = [nc.sync, nc.scalar, nc.gpsimd]
        # wv per-k chunks + xt t=0 first (V(t=0) needs wv[k] and xt[t0])
        nc.sync.dma_start(wv_big[:, 0:2 * F], wvt_d[:, 0:2 * F])
        nc.scalar.dma_start(xt_big[:, 0:KJ], xt_d[:, 0:KJ])
        nc.gpsimd.dma_start(wv_big[:, 2 * F:4 * F], wvt_d[:, 2 * F:4 * F])
        nc.sync.dma_start(xt_big[:, KJ:2 * KJ], xt_d[:, KJ:2 * KJ])
        nc.scalar.dma_start(wv_big[:, 4 * F:6 * F], wvt_d[:, 4 * F:6 * F])
        nc.gpsimd.dma_start(wv_big[:, 6 * F:8 * F], wvt_d[:, 6 * F:8 * F])
        for t in range(2, NTB):
            engs[t % 3].dma_start(xt_big[:, t * KJ:(t + 1) * KJ],
                                  xt_d[:, t * KJ:(t + 1) * KJ])
        for h in range(2):
            hw = slice(h * 4 * F, (h + 1) * 4 * F)
            nc.sync.dma_start(wq_big[:, hw], wqt_d[:, hw])
            nc.scalar.dma_start(wk_big[:, hw], wkt_d[:, hw])
        nc.gpsimd.dma_start(cs_sb[:], cs_d)
        nc.gpsimd.dma_start(ss_sb[:], ss_d)
        nc.sync.dma_start(bq_sb[:], bq_d)

        swp = top.enter_context(tc.tile_pool(name="swp", bufs=2))
        ptp = top.enter_context(tc.tile_pool(name="ptp", bufs=18))
        rbp = top.enter_context(tc.tile_pool(name="rbp", bufs=2))
        ost = top.enter_context(tc.tile_pool(name="ost", bufs=3))

        pq = top.enter_context(tc.tile_pool(name="pq", bufs=1, space="PSUM"))

        def qk_full(m, which):
            """Full Q or K projection for pair m: 16 MMs, halves alternating."""
            w_sb = wq_sb if which == "q" else wk_sb
            ps = pq.tile([P, N], f32, tag="pq", name="psqk")
            for k in range(NKB):
                for half in range(2):
                    hs = slice(half * F, (half + 1) * F)
                    nc.tensor.matmul(
                        ps[:, hs], w_sb[k][:, m * P:(m + 1) * P],
                        xt4[:, half * 4:half * 4 + 4, k, :],
                        start=(k == 0), stop=(k == NKB - 1))
            return ps

        def qk_store(m, which, ps):
            dst = qt_sb if which == "q" else kt_sb
            if which == "q":
                nc.vector.tensor_scalar_add(dst[m][:], ps[:], bq_sb[:, m:m + 1])
            else:
                nc.vector.tensor_copy(dst[m][:], ps[:])

        def rotary(m, src_sb):
            sw = swp.tile([P, N], dt, tag="sw", name="sw")
            for blk in range(4):
                o = blk * 32
                nc.gpsimd.dma_start(sw[o:o + 32, :],
                                    src_sb[m][o ^ 32:(o ^ 32) + 32, :])
            nc.vector.tensor_tensor(sw[:], sw[:], ss_sb[:], op=ALU.mult)
            nc.vector.tensor_tensor(src_sb[m][:], src_sb[m][:], cs_sb[:],
                                    op=ALU.mult)
            nc.vector.tensor_tensor(src_sb[m][:], src_sb[m][:], sw[:],
                                    op=ALU.add)

        def norm_head(m, par, av):
            """reciprocal of ones-row + half-broadcast (issue right after
            the AV stop so gpsimd works while vector does other things)."""
            h = 2 * m + par
            nc.vector.reciprocal_approx_fast(rcp_h[h][:], av[0:1, :])
            rb = rbp.tile([64, N], f32, tag="rb", name="rb")
            nc.gpsimd.partition_broadcast(rb[:], rcp_h[h][:])
            return rb

        def norm_tail(m, par, av, rb):
            """scale psum -> ot (drains the AV psum)."""
            off = par * 64
            nc.vector.tensor_tensor(ot_sb[m][off:off + 64, :], av[64:128, :],
                                    rb[:], op=ALU.mult)

        def norm_split(m, par, av):
            """pipelined half-column variant for the latency-critical last
            chains: recip/bcast/mult on 512-col halves."""
            h = 2 * m + par
            off = par * 64
            rbs = []
            for c in range(2):
                cs2 = slice(c * F, (c + 1) * F)
                nc.vector.reciprocal_approx_fast(rcp_h[h][:, cs2], av[0:1, cs2])
                rb = rbp.tile([64, F], f32, tag="rbs", name="rbs")
                nc.gpsimd.partition_broadcast(rb[:], rcp_h[h][:, cs2])
                rbs.append(rb)
            for c in range(2):
                cs2 = slice(c * F, (c + 1) * F)
                nc.vector.tensor_tensor(ot_sb[m][off:off + 64, cs2],
                                        av[64:128, cs2], rbs[c][:],
                                        op=ALU.mult)

        # ---------- phase 0: V projections; QK(0) threaded into t=5..7 ----
        with ExitStack() as ph0:
            pv = ph0.enter_context(tc.tile_pool(name="pv", bufs=4, space="PSUM"))
            psq0 = psk0 = None
            for t in range(NTB):
                ps = pv.tile([P, F], f32, tag="pv", name="psv")
                for k in range(NKB):
                    nc.tensor.matmul(
                        ps[:], xt4[:, t, k, :], wv_sb[k][:],
                        start=(k == 0), stop=(k == NKB - 1))
                dst = v65_sb[t][:].rearrange("p (h e) -> p h e", e=128)[:, :, 64:128]
                nc.vector.tensor_copy(dst,
                                      ps[:].rearrange("p (h e) -> p h e", e=64))
                if t == 5:
                    psq0 = qk_full(0, "q")
                elif t == 6:
                    qk_store(0, "q", psq0)
                    psk0 = qk_full(0, "k")
                    rotary(0, qt_sb)
                elif t == 7:
                    qk_store(0, "k", psk0)
                    rotary(0, kt_sb)
        nc.sync.dma_start(wot_big[:], wot_d)

        # ---------- attention pairs ----------
        with ExitStack() as phc:
            pss = phc.enter_context(tc.tile_pool(name="pss", bufs=2, space="PSUM"))
            pav = phc.enter_context(tc.tile_pool(name="pav", bufs=1, space="PSUM"))
            for m in range(4):
                nxt = m + 1
                last = nxt >= 4
                pts = ([], [])
                psq = None if last else pq.tile([P, N], f32, tag="pq", name="psq")
                av0 = av1 = None
                for kb in range(NTB):
                    ps2 = [pss.tile([P, N], f32, tag="pss", name="pss")
                           for _ in range(2)]
                    for qh in range(2):
                        qs = slice(qh * F, (qh + 1) * F)
                        for par in range(2):
                            off = par * 64
                            nc.tensor.matmul(
                                ps2[par][:, qs],
                                kt_sb[m][off:off + 64, kb * P:(kb + 1) * P],
                                qt_sb[m][off:off + 64, qs],
                                start=True, stop=True,
                                tile_position=(off, 0))
                    if not last:
                        # thread next pair's Q projection: 2 MMs per kb
                        for half in range(2):
                            hs = slice(half * F, (half + 1) * F)
                            nc.tensor.matmul(
                                psq[:, hs], wq_sb[kb][:, nxt * P:(nxt + 1) * P],
                                xt4[:, half * 4:half * 4 + 4, kb, :],
                                start=(kb == 0), stop=(kb == NKB - 1))
                    else:
                        # last pair: interleave AV par0 (lag 1, pav psum) and
                        # AV par1 (lag 2, borrowed pq psum) into the loop
                        if kb == 1:
                            av0 = pav.tile([P, N], f32, tag="pav", name="av")
                        if kb >= 1:
                            j = kb - 1
                            for qh in range(2):
                                qs = slice(qh * F, (qh + 1) * F)
                                nc.tensor.matmul(
                                    av0[:, qs],
                                    v65_sb[j][:, (2 * m) * 128:(2 * m + 1) * 128],
                                    pts[0][j][:, qs],
                                    start=(j == 0), stop=False)
                        if kb == 2:
                            av1 = pq.tile([P, N], f32, tag="pq", name="av1")
                        if kb >= 2:
                            j = kb - 2
                            for qh in range(2):
                                qs = slice(qh * F, (qh + 1) * F)
                                nc.tensor.matmul(
                                    av1[:, qs],
                                    v65_sb[j][:, (2 * m + 1) * 128:(2 * m + 2) * 128],
                                    pts[1][j][:, qs],
                                    start=(j == 0), stop=False)
                    for par in range(2):
                        ptile = ptp.tile([P, N], dt, tag="pt", name="ptile",
                                         bufs=18)
                        nc.scalar.activation(ptile[:], ps2[par][:], AF.Exp,
                                             scale=0.125, bias=ebias[:])
                        pts[par].append(ptile)

                if not last:
                    qk_store(nxt, "q", psq)
                    rotary(nxt, qt_sb)
                    # AV par0
                    av0 = pav.tile([P, N], f32, tag="pav", name="av")
                    h0 = 2 * m
                    for kb in range(NTB):
                        for qh in range(2):
                            qs = slice(qh * F, (qh + 1) * F)
                            nc.tensor.matmul(
                                av0[:, qs],
                                v65_sb[kb][:, h0 * 128:(h0 + 1) * 128],
                                pts[0][kb][:, qs],
                                start=(kb == 0), stop=(kb == NTB - 1))
                    # chain0 intact on vector: recip, bcast (gpsimd), mult.
                    # It completes under the K-projection MMs.
                    rb0 = norm_head(m, 0, av0)
                    norm_tail(m, 0, av0, rb0)
                    # K(m+1) projection: 16 MMs cover chain0 + psum drain
                    psk = qk_full(nxt, "k")
                    # kt copy + rotary k run on vector/gpsimd during AV par1
                    qk_store(nxt, "k", psk)
                    rotary(nxt, kt_sb)
                    # AV par1 (pav rotation waits chain0's mult - covered)
                    av1 = pav.tile([P, N], f32, tag="pav", name="av")
                    h1 = 2 * m + 1
                    for kb in range(NTB):
                        for qh in range(2):
                            qs = slice(qh * F, (qh + 1) * F)
                            nc.tensor.matmul(
                                av1[:, qs],
                                v65_sb[kb][:, h1 * 128:(h1 + 1) * 128],
                                pts[1][kb][:, qs],
                                start=(kb == 0), stop=(kb == NTB - 1))
                    rb1 = norm_head(m, 1, av1)
                    norm_tail(m, 1, av1, rb1)
                else:
                    # finish AV par0 (kb 7), chain0 while par1 finishes
                    h0, h1 = 2 * m, 2 * m + 1
                    for qh in range(2):
                        qs = slice(qh * F, (qh + 1) * F)
                        nc.tensor.matmul(
                            av0[:, qs],
                            v65_sb[7][:, h0 * 128:(h0 + 1) * 128],
                            pts[0][7][:, qs], start=False, stop=True)
                    norm_split(m, 0, av0)
                    for j in (6, 7):
                        for qh in range(2):
                            qs = slice(qh * F, (qh + 1) * F)
                            nc.tensor.matmul(
                                av1[:, qs],
                                v65_sb[j][:, h1 * 128:(h1 + 1) * 128],
                                pts[1][j][:, qs], start=False, stop=(j == 7))
                    norm_split(m, 1, av1)

        # ---------- o_proj: pairs 0..2 first, pair 3 contributions last ----
        def oproj_fin(ps, qb):
            for half in range(2):
                hs = slice(half * F, (half + 1) * F)
                nc.tensor.matmul(ps[:, hs], ot_sb[3][:, qb * P:(qb + 1) * P],
                                 wot_sb[3][:, hs], start=False, stop=True)
            ot_out = ost.tile([P, D], dt, tag="oout", name="oout")
            if qb % 2 == 0:
                nc.scalar.copy(ot_out[:], ps[:])
                nc.sync.dma_start(out_d[qb * P:(qb + 1) * P, :], ot_out[:])
            else:
                nc.vector.tensor_copy(ot_out[:], ps[:])
                nc.scalar.dma_start(out_d[qb * P:(qb + 1) * P, :], ot_out[:])

        with ExitStack() as phe:
            po = phe.enter_context(tc.tile_pool(name="po", bufs=3, space="PSUM"))
            pso = []
            for qb in range(NTB):
                ps = po.tile([P, D], f32, tag="po", name="pso", bufs=3)
                pso.append(ps)
                for mm_ in range(3):
                    for half in range(2):
                        hs = slice(half * F, (half + 1) * F)
                        nc.tensor.matmul(
                            ps[:, hs],
                            ot_sb[mm_][:, qb * P:(qb + 1) * P],
                            wot_sb[mm_][:, hs],
                            start=(mm_ == 0), stop=False)
                if qb >= 2:
                    oproj_fin(pso[qb - 2], qb - 2)
            for qb in range(NTB - 2, NTB):
                oproj_fin(pso[qb], qb)

    nc.compile()
    return nc


def host_prep(inputs, mode=MODE):
    """Slice/transpose full inputs into 8 per-core input maps."""
    hs = np.asarray(inputs["hidden_states"], np.float32)
    cos = np.asarray(inputs["cos"], np.float32)
    sin = np.asarray(inputs["sin"], np.float32)
    wq = np.asarray(inputs["wq"], np.float32)
    wk = np.asarray(inputs["wk"], np.float32)
    wv = np.asarray(inputs["wv"], np.float32)
    wo = np.asarray(inputs["wo"], np.float32)
    bq = np.asarray(inputs["bq"], np.float32)

    if mode == "bf16":
        import ml_dtypes
        cast = lambda a: np.ascontiguousarray(a).astype(ml_dtypes.bfloat16)
    elif mode == "f16":
        cast = lambda a: np.ascontiguousarray(a).astype(np.float16)
    else:
        cast = lambda a: np.ascontiguousarray(a, np.float32)

    sgn = np.ones((64, 1), np.float32)
    sgn[:32] = -1.0
    in_maps = []
    for c in range(8):
        b, g = c // 2, c % 2
        fs = slice(g * F, (g + 1) * F)
        csx = cos[b].T  # (64, N)
        ssx = sin[b].T * sgn
        pack = lambda a, kd: np.ascontiguousarray(
            a.reshape(kd, P, -1).transpose(1, 0, 2).reshape(P, -1))
        # xt t-major: [P, t, k, j] from hs[b].T = [k*P+p, t*P+j]
        xtm = hs[b].T.reshape(NKB, P, NTB, P).transpose(1, 2, 0, 3).reshape(P, -1)
        in_maps.append({
            "xt": cast(np.ascontiguousarray(xtm)),
            "wqt": cast(pack(wq[fs, :].T, NKB)),
            "wkt": cast(pack(wk[fs, :].T, NKB)),
            "wvt": cast(pack(wv[fs, :].T, NKB)),
            "wot": cast(pack(wo[:, fs].T, 4)),
            "bq": np.ascontiguousarray(bq[fs].reshape(4, P).T, np.float32),
            "cs": cast(np.concatenate([csx, csx], axis=0)),
            "ss": cast(np.concatenate([ssx, ssx], axis=0)),
        })
    return in_maps


def host_finish(results, inputs):
    bo = np.asarray(inputs["bo"], np.float32)
    bv = np.asarray(inputs["bv"], np.float32)
    wo = np.asarray(inputs["wo"], np.float32)
    const = bo + bv @ wo.T
    out = np.empty((B, N, D), np.float32)
    for b in range(B):
        out[b] = (results[2 * b]["out"].astype(np.float32)
                  + results[2 * b + 1]["out"].astype(np.float32) + const)
    return out


def _get_nc(mode=MODE):
    if mode not in _CACHE:
        _CACHE[mode] = build_nc(mode)
    return _CACHE[mode]


def run(inputs, mode=MODE, trace=False, tmpdir=None):
    nc = _get_nc(mode)
    in_maps = host_prep(inputs, mode)
    res = bass_utils.run_bass_kernel_spmd(
        nc, in_maps, core_ids=list(range(8)), trace=trace, tmpdir=tmpdir)
    return host_finish(res.results, inputs), res


def kernel(**inputs):
    out, _ = run(inputs)
    return out
